# revision 2
# baseline (speedup 1.0000x reference)
"""DCNv2 (modulated deformable conv) Trainium2 Bass kernel.

Sharding: 8 cores = (batch b in 2) x (H-quarter q in 4); each core computes
out[b, :, 16q:16q+16, :] (256 out-channels x 1024 pixels).

Algorithm per core (v2):
  1. offset/mask 3x3 conv on an 18-row slab -> om [27, 1024] (bf16 PE matmuls).
  2. om repacked [27,1024] -> [108, 256] via DRAM bounce so geometry runs on
     ~108 partitions instead of 9 (8x fewer DVE cycles).
  3. geometry: ys/xs, floor via magic, fractions, sigmoid mask; lambda planes
     lx | lam0=m*(1-ly) | lam1=m*ly in one [36, 3, 256] fp16 tile; gather
     indices written to DRAM PRE-WRAPPED ([16, 576] int16) so the index
     readback is contiguous (the old strided readback burned ~30us of DMA).
  4. per tap: ONE dma_gather (1024 idx, 2KB rows) from a host-built "pairs"
     table row = [A0|A1|B0|B1] (A=value at y0/y0+1, B=x-neighbor diff).
  5. combine in 3 DVE ops/tap: t = lx*B(4 planes), R = A + t,
     v = [lam0,lam0,lam1,lam1] * R  -> 512-channel GEMM rhs.
  6. main conv: osum[o, p] += sum_{tap, 4 vblocks} wdcnT^T @ v (fp16 PE).
"""
import os
import sys
import numpy as np

sys.path.insert(0, "/opt/trn_rl_repo")

from contextlib import ExitStack
import ml_dtypes
import concourse.bass as bass
import concourse.bacc as bacc
import concourse.tile as tile
import concourse.mybir as mybir
from concourse.bass_utils import run_bass_kernel_spmd


AF = mybir.ActivationFunctionType
ALU = mybir.AluOpType
DT = mybir.dt

B, C, H, W, K2 = 2, 256, 64, 64, 9
PADG = 6
G = H + 2 * PADG            # 76 padded grid side
NROWS = G * G               # 5776 table rows
HQ = 16                     # output rows per core
T = HQ * W                  # 1024 pixels per core
N_CORES = 8
MAGIC = 12582912.0          # 1.5 * 2^23 round-to-int magic

_cache = {}


def _build_module():
    nc = bacc.Bacc("TRN2", debug=False, num_devices=N_CORES,
                   dynamic_dma_scratch_size=131072)

    # ---- dram tensors -------------------------------------------------------
    slab = nc.dram_tensor("slab", [C, 18, 66], DT.bfloat16, kind="ExternalInput")
    womT = nc.dram_tensor("womT", [18, 128, 27], DT.bfloat16, kind="ExternalInput")
    bom = nc.dram_tensor("bom", [27, 1], DT.float32, kind="ExternalInput")
    wdT = nc.dram_tensor("wdT", [18, 128, 256], DT.float16, kind="ExternalInput")
    bdcn = nc.dram_tensor("bdcn", [128, 2], DT.float32, kind="ExternalInput")
    baseyx = nc.dram_tensor("baseyx", [72, 256], DT.float32, kind="ExternalInput")
    table = nc.dram_tensor("table", [NROWS, 1024], DT.float16, kind="ExternalInput")
    sel = nc.dram_tensor("sel", [36, 36 * 128], DT.float16, kind="ExternalInput")
    out = nc.dram_tensor("out", [C, T], DT.float32, kind="ExternalOutput")
    ibounce = nc.dram_tensor("ibounce", [16, 576], DT.int16)
    obounce = nc.dram_tensor("obounce", [108, 256], DT.float32)

    with tile.TileContext(nc) as tc, ExitStack() as ctx:
        consts = ctx.enter_context(tc.tile_pool(name="consts", bufs=1))
        geom = ctx.enter_context(tc.tile_pool(name="geom", bufs=1))
        plpsum = ctx.enter_context(tc.tile_pool(name="plpsum", bufs=2,
                                                space="PSUM"))
        opsum = ctx.enter_context(tc.tile_pool(name="opsum", bufs=1,
                                               space="PSUM"))
        planes = ctx.enter_context(tc.tile_pool(name="planes", bufs=2))
        gbuf = ctx.enter_context(tc.tile_pool(name="gbuf", bufs=2))
        cbuf = ctx.enter_context(tc.tile_pool(name="cbuf", bufs=2))

        # ---- load constants -------------------------------------------------
        t_slab = []
        for ch in range(2):
            s = consts.tile([128, 18, 66], DT.bfloat16, name=f"slab{ch}")
            nc.sync.dma_start(s[:], slab.ap()[128 * ch:128 * (ch + 1)])
            t_slab.append(s)
        t_womT = consts.tile([128, 18, 27], DT.bfloat16)
        nc.sync.dma_start(t_womT[:], womT.ap().transpose([1, 0, 2]))
        t_wdT = consts.tile([128, 18, 256], DT.float16)
        nc.sync.dma_start(t_wdT[:], wdT.ap().transpose([1, 0, 2]))
        t_bom = consts.tile([27, 1], DT.float32)
        nc.sync.dma_start(t_bom[:], bom.ap())
        t_bdcn = consts.tile([128, 2], DT.float32)
        nc.sync.dma_start(t_bdcn[:], bdcn.ap())
        t_baseyx = consts.tile([72, 256], DT.float32)
        nc.sync.dma_start(t_baseyx[:], baseyx.ap())
        t_sel = consts.tile([36, 36 * 128], DT.float16)
        nc.sync.dma_start(t_sel[:], sel.ap())

        # ---- offset conv: om27 rows = [dy(9) | dx(9) | m(9)] ---------------
        t_om27 = geom.tile([27, T], DT.float32, name="om27")
        for nh in range(2):
            ps = plpsum.tile([27, 512], DT.float32, tag="pl")
            i = 0
            for k in range(K2):
                ky, kx = k // 3, k % 3
                for ch in range(2):
                    rhs = t_slab[ch][:, 8 * nh + ky: 8 * nh + ky + 8, kx:kx + 64]
                    nc.tensor.matmul(ps[:], t_womT[:, 2 * k + ch], rhs,
                                     start=(i == 0), stop=(i == 17))
                    i += 1
            nc.scalar.activation(t_om27[:, 512 * nh:512 * (nh + 1)], ps[:],
                                 AF.Identity, bias=t_bom[:])

        # repack [27, 1024] -> [108, 256] via DRAM bounce (row p = flat 256p)
        nc.sync.dma_start(obounce.ap(), t_om27[:])
        t_omp = geom.tile([108, 256], DT.float32, name="omp")
        nc.sync.dma_start(t_omp[:], obounce.ap())

        # ---- geometry (packed [*, 256], fp32) -------------------------------
        t_ysxs = geom.tile([72, 256], DT.float32, name="ysxs")
        nc.vector.tensor_add(t_ysxs[:], t_omp[0:72], t_baseyx[:])
        t_r = geom.tile([72, 256], DT.float32, name="rtmp")
        nc.vector.tensor_scalar(t_r[:], t_ysxs[:], MAGIC, -MAGIC, ALU.add, ALU.add)
        t_gt = geom.tile([72, 256], DT.float32, name="gttmp")
        nc.vector.tensor_tensor(t_gt[:], t_r[:], t_ysxs[:], ALU.is_gt)
        t_fl = geom.tile([72, 256], DT.float32, name="fl")
        nc.vector.tensor_sub(t_fl[:], t_r[:], t_gt[:])

        # gather index first (unblocks the gathers asap):
        # idx = clip(y0+PADG, 0, G-1)*G + clip(x0+PADG, 0, G-1)
        t_yp = geom.tile([36, 256], DT.float32, name="yp")
        nc.vector.tensor_scalar(t_yp[:], t_fl[0:36], float(PADG), 0.0,
                                ALU.add, ALU.max)
        t_ypg = geom.tile([36, 256], DT.float32, name="ypg")
        nc.vector.tensor_scalar(t_ypg[:], t_yp[:], float(G - 1), float(G),
                                ALU.min, ALU.mult)
        t_xp = geom.tile([36, 256], DT.float32, name="yp")
        nc.vector.tensor_scalar(t_xp[:], t_fl[36:72], float(PADG), 0.0,
                                ALU.add, ALU.max)
        t_xpc = geom.tile([36, 256], DT.float32, name="ypg2")
        nc.vector.tensor_scalar(t_xpc[:], t_xp[:], float(G - 1), None, ALU.min)
        t_idx = geom.tile([36, 256], DT.float32, name="idx")
        nc.vector.tensor_add(t_idx[:], t_ypg[:], t_xpc[:])

        # export pre-wrapped: ibounce[b, p*16 + a] = idx[p, a*16 + b]
        # (pixel i = qq*256 + a*16 + b; lane = i%16 = b, slot = i//16 = qq*16+a)
        nc.gpsimd.dma_start(
            bass.AP(ibounce, 0, [[16, 36], [576, 16], [1, 16]]),
            t_idx[:].rearrange("p (a b) -> p b a", a=16, b=16))
        t_widx = consts.tile([128, 576], DT.int16)
        for g in range(8):
            nc.sync.dma_start(t_widx[16 * g:16 * (g + 1), :], ibounce.ap())

        # lambda planes [36, 3, 256] fp16: lx | lam0=m-m*ly | lam1=m*ly
        t_fr = geom.tile([72, 256], DT.float32, name="fr")
        nc.vector.tensor_sub(t_fr[:], t_ysxs[:], t_fl[:])
        t_mask = geom.tile([36, 256], DT.float32, name="mask")
        nc.scalar.activation(t_mask[:], t_omp[72:108], AF.Sigmoid)
        t_lam = geom.tile([36, 3, 256], DT.float16, name="lam")
        nc.vector.tensor_copy(t_lam[:, 0], t_fr[36:72])
        nc.vector.tensor_mul(t_lam[:, 2], t_mask[:], t_fr[0:36])
        nc.vector.tensor_sub(t_lam[:, 1], t_mask[:], t_lam[:, 2])

        # ---- per-tap: gather, planes, combine, matmul -----------------------
        t_osum = [opsum.tile([128, T], DT.float32, name=f"osum{oh}")
                  for oh in range(2)]
        for k in range(K2):
            gc = gbuf.tile([128, 8, T], DT.float16, name="gath")
            nc.gpsimd.dma_gather(
                out_ap=gc[:],
                in_ap=table.ap(),
                idxs_ap=t_widx[:, 64 * k:64 * (k + 1)],
                num_idxs=T,
                num_idxs_reg=T,
                elem_size=1024,
                transpose=True,
            )

            pl = planes.tile([128, 3, T], DT.float16, name="plane")
            for qq in range(4):
                pp = plpsum.tile([128, 3, 256], DT.float32, tag="pl")
                nc.tensor.matmul(pp[:],
                                 t_sel[:, 128 * (4 * k + qq):
                                       128 * (4 * k + qq + 1)],
                                 t_lam[:], start=True, stop=True)
                nc.scalar.activation(pl[:, :, 256 * qq:256 * (qq + 1)], pp[:],
                                     AF.Copy)

            # t = lx * B; R = A + t; v = [l0,l0,l1,l1] * R
            tt = cbuf.tile([128, 4, T], DT.float16, name="tt")
            nc.vector.tensor_mul(
                tt[:], gc[:, 4:8],
                pl[:, 0].unsqueeze(1).broadcast_to([128, 4, T]))
            rr = cbuf.tile([128, 4, T], DT.float16, name="rr")
            nc.vector.tensor_add(rr[:], gc[:, 0:4], tt[:])
            vv = cbuf.tile([128, 4, T], DT.float16, name="vv")
            nc.vector.tensor_mul(
                vv[:].rearrange("p (r h) f -> p r h f", r=2, h=2),
                rr[:].rearrange("p (r h) f -> p r h f", r=2, h=2),
                pl[:, 1:3].unsqueeze(2).broadcast_to([128, 2, 2, T]))

            for vb in range(4):
                lhsT = t_wdT[:, 2 * k + (vb % 2)]       # [128, 256]
                for oh in range(2):
                    for nh in range(2):
                        nc.tensor.matmul(
                            t_osum[oh][:, 512 * nh:512 * (nh + 1)],
                            lhsT[:, 128 * oh:128 * (oh + 1)],
                            vv[:, vb, 512 * nh:512 * (nh + 1)],
                            start=(k == 0 and vb == 0),
                            stop=(k == K2 - 1 and vb == 3),
                        )

        # ---- output ---------------------------------------------------------
        for oh in range(2):
            osb = cbuf.tile([128, T], DT.float32, name="osb")
            nc.scalar.activation(osb[:], t_osum[oh][:], AF.Identity,
                                 bias=t_bdcn[:, oh:oh + 1])
            nc.sync.dma_start(out.ap()[128 * oh:128 * (oh + 1)], osb[:])

    nc.compile()
    return nc


def _host_prep(x, offset_feat, w_offset_mask, b_offset_mask, w_dcn, b_dcn):
    bf16 = ml_dtypes.bfloat16
    perm = list(range(0, 18, 2)) + list(range(1, 18, 2)) + list(range(18, 27))
    w_om_p = w_offset_mask[perm].astype(np.float32)      # [27, 256, 3, 3]
    b_om_p = b_offset_mask[perm].astype(np.float32)

    # womT[k*2+ch] = [128 c, 27] for tap k, channel half ch
    womT = np.zeros((18, 128, 27), np.float32)
    for k in range(9):
        ky, kx = k // 3, k % 3
        wt = w_om_p[:, :, ky, kx]                        # [27, 256]
        for ch in range(2):
            womT[2 * k + ch] = wt[:, 128 * ch:128 * (ch + 1)].T
    womT = womT.astype(bf16)

    # wdT[k*2+ch] = [128 c, 256 o]
    wd = w_dcn.reshape(C, C, 9).astype(np.float32)       # [o, c, k]
    wdT = np.zeros((18, 128, 256), np.float32)
    for k in range(9):
        for ch in range(2):
            wdT[2 * k + ch] = wd[:, 128 * ch:128 * (ch + 1), k].T
    wdT = wdT.astype(np.float16)

    bdcn = b_dcn.astype(np.float32).reshape(2, 128).T.copy()  # [128, 2]

    # pairs tables per batch: row = [A0 | A1 | B0 | B1]
    tables = []
    for b in range(B):
        xp = np.zeros((C, G + 1, G + 1), np.float32)
        xp[:, PADG:PADG + H, PADG:PADG + W] = x[b]
        A0 = xp[:, :G, :G]
        A1 = xp[:, 1:, :G]
        B0 = xp[:, :G, 1:] - A0
        B1 = xp[:, 1:, 1:] - A1
        tab = np.concatenate(
            [t.reshape(C, NROWS).T for t in (A0, A1, B0, B1)], axis=1)
        tables.append(np.ascontiguousarray(tab.astype(np.float16)))

    # packed base grids [72, 256] per h-quarter: rows (4k+qq)
    ky = np.repeat(np.arange(3), 3).astype(np.float32)   # [9]
    kx = np.tile(np.arange(3), 3).astype(np.float32)
    f = np.arange(256)
    selv = np.zeros((36, 36 * 128), np.float16)
    for r in range(36):
        selv[r, 128 * r:128 * (r + 1)] = 1.0
    in_maps = []
    for b in range(B):
        for q in range(4):
            ho0 = q * HQ
            slab = np.zeros((C, 18, 66), np.float32)
            r0 = ho0 - 1
            rr0, rr1 = max(r0, 0), min(ho0 + 17, H)
            slab[:, rr0 - r0:rr1 - r0, 1:65] = offset_feat[b][:, rr0:rr1, :]
            byx = np.zeros((72, 256), np.float32)
            for k in range(9):
                for qq in range(4):
                    i = qq * 256 + f
                    byx[4 * k + qq] = ho0 + i // 64 + ky[k] - 1.0
                    byx[36 + 4 * k + qq] = i % 64 + kx[k] - 1.0
            in_maps.append({
                "slab": slab.astype(bf16),
                "womT": womT,
                "bom": b_om_p.reshape(27, 1),
                "wdT": wdT,
                "bdcn": bdcn,
                "baseyx": byx,
                "table": tables[b],
                "sel": selv,
            })
    return in_maps


def _get_module():
    if "nc" not in _cache:
        _cache["nc"] = _build_module()
    return _cache["nc"]


def kernel(x, offset_feat, w_offset_mask, b_offset_mask, w_dcn, b_dcn,
           **run_kwargs):
    x = np.asarray(x); offset_feat = np.asarray(offset_feat)
    w_offset_mask = np.asarray(w_offset_mask)
    b_offset_mask = np.asarray(b_offset_mask)
    w_dcn = np.asarray(w_dcn); b_dcn = np.asarray(b_dcn)

    nc = _get_module()
    in_maps = _host_prep(x, offset_feat, w_offset_mask, b_offset_mask,
                         w_dcn, b_dcn)
    res = run_bass_kernel_spmd(nc, in_maps, list(range(N_CORES)), **run_kwargs)
    y = np.zeros((B, C, H, W), np.float32)
    for ci, (b, q) in enumerate([(b, q) for b in range(B) for q in range(4)]):
        y[b, :, q * HQ:(q + 1) * HQ, :] = \
            res.results[ci]["out"].reshape(C, HQ, W)
    kernel.last_results = res
    return y


# revision 18
# speedup vs baseline: 2.7779x; 2.7779x over previous
"""DCNv2 (modulated deformable conv) Trainium2 Bass kernel.

Sharding: 8 cores = (batch b in 2) x (H-quarter q in 4); each core computes
out[b, :, 16q:16q+16, :] (256 out-channels x 1024 pixels).

Algorithm per core (v2):
  1. offset/mask 3x3 conv on an 18-row slab -> om [27, 1024] (bf16 PE matmuls).
  2. om repacked [27,1024] -> [108, 256] via DRAM bounce so geometry runs on
     ~108 partitions instead of 9 (8x fewer DVE cycles).
  3. geometry: ys/xs, floor via magic, fractions, sigmoid mask; lambda planes
     lx | lam0=m*(1-ly) | lam1=m*ly in one [36, 3, 256] fp16 tile; gather
     indices written to DRAM PRE-WRAPPED ([16, 576] int16) so the index
     readback is contiguous (the old strided readback burned ~30us of DMA).
  4. per tap: ONE dma_gather (1024 idx, 2KB rows) from a host-built "pairs"
     table row = [A0|A1|B0|B1] (A=value at y0/y0+1, B=x-neighbor diff).
  5. combine in 3 DVE ops/tap: t = lx*B(4 planes), R = A + t,
     v = [lam0,lam0,lam1,lam1] * R  -> 512-channel GEMM rhs.
  6. main conv: osum[o, p] += sum_{tap, 4 vblocks} wdcnT^T @ v (fp16 PE).
"""
import os
import sys
import numpy as np

sys.path.insert(0, "/opt/trn_rl_repo")

from contextlib import ExitStack
import ml_dtypes
import concourse.bass as bass
import concourse.bacc as bacc
import concourse.tile as tile
import concourse.mybir as mybir
from concourse.bass_utils import run_bass_kernel_spmd


AF = mybir.ActivationFunctionType
ALU = mybir.AluOpType
DT = mybir.dt

B, C, H, W, K2 = 2, 256, 64, 64, 9
PADG = 6
G = H + 2 * PADG            # 76 padded grid side
NROWS = G * G               # 5776 table rows
HQ = 16                     # output rows per core
T = HQ * W                  # 1024 pixels per core
N_CORES = 8
MAGIC = 12582912.0          # 1.5 * 2^23 round-to-int magic

_cache = {}


def _build_module():
    nc = bacc.Bacc("TRN2", debug=False, num_devices=N_CORES,
                   dynamic_dma_scratch_size=49152)

    # ---- dram tensors -------------------------------------------------------
    slab = nc.dram_tensor("slab", [C, 18, 66], DT.bfloat16, kind="ExternalInput")
    womT = nc.dram_tensor("womT", [18, 128, 27], DT.bfloat16, kind="ExternalInput")
    bom = nc.dram_tensor("bom", [27, 1], DT.float32, kind="ExternalInput")
    wdT = nc.dram_tensor("wdT", [18, 128, 256], DT.float16, kind="ExternalInput")
    bdcn = nc.dram_tensor("bdcn", [128, 2], DT.float32, kind="ExternalInput")
    baseyx = nc.dram_tensor("baseyx", [36, 2, 256], DT.float32, kind="ExternalInput")
    table = nc.dram_tensor("table", [NROWS, 1024], DT.float16, kind="ExternalInput")
    sel = nc.dram_tensor("sel", [36, 36 * 128], DT.float16, kind="ExternalInput")
    out = nc.dram_tensor("out", [C, T], DT.float32, kind="ExternalOutput")
    ibounce = nc.dram_tensor("ibounce", [16, 576], DT.int16)
    obounce = nc.dram_tensor("obounce", [108, 256], DT.float32)

    with tile.TileContext(nc) as tc, ExitStack() as ctx:
        consts = ctx.enter_context(tc.tile_pool(name="consts", bufs=1))
        geom = ctx.enter_context(tc.tile_pool(name="geom", bufs=1))
        plpsum = ctx.enter_context(tc.tile_pool(name="plpsum", bufs=2,
                                                space="PSUM"))
        opsum = ctx.enter_context(tc.tile_pool(name="opsum", bufs=1,
                                               space="PSUM"))
        planes = ctx.enter_context(tc.tile_pool(name="planes", bufs=2))
        gbuf = ctx.enter_context(tc.tile_pool(name="gbuf", bufs=3))
        cbuf = ctx.enter_context(tc.tile_pool(name="cbuf", bufs=2))

        # ---- load constants -------------------------------------------------
        t_slab = []
        for ch in range(2):
            s = consts.tile([128, 18, 66], DT.bfloat16, name=f"slab{ch}")
            nc.sync.dma_start(s[:], slab.ap()[128 * ch:128 * (ch + 1)])
            t_slab.append(s)
        t_womT = consts.tile([128, 18, 27], DT.bfloat16)
        nc.sync.dma_start(t_womT[:], womT.ap().transpose([1, 0, 2]))
        t_wdT = consts.tile([128, 18, 256], DT.float16)
        nc.sync.dma_start(t_wdT[:], wdT.ap().transpose([1, 0, 2]))
        t_bom = consts.tile([27, 1], DT.float32)
        nc.sync.dma_start(t_bom[:], bom.ap())
        t_bdcn = consts.tile([128, 2], DT.float32)
        nc.sync.dma_start(t_bdcn[:], bdcn.ap())
        t_baseyx = consts.tile([36, 2, 256], DT.float32)
        nc.sync.dma_start(t_baseyx[:], baseyx.ap())
        t_sel = consts.tile([36, 36 * 128], DT.float16)
        nc.sync.dma_start(t_sel[:], sel.ap())

        # ---- offset conv: om27 rows = [dy(9) | dx(9) | m(9)] ---------------
        t_om27 = geom.tile([27, T], DT.float32, name="om27")
        for nh in range(2):
            ps = plpsum.tile([27, 512], DT.float32, tag="pl")
            i = 0
            for k in range(K2):
                ky, kx = k // 3, k % 3
                for ch in range(2):
                    rhs = t_slab[ch][:, 8 * nh + ky: 8 * nh + ky + 8, kx:kx + 64]
                    nc.tensor.matmul(ps[:], t_womT[:, 2 * k + ch], rhs,
                                     start=(i == 0), stop=(i == 17))
                    i += 1
            nc.scalar.activation(t_om27[:, 512 * nh:512 * (nh + 1)], ps[:],
                                 AF.Identity, bias=t_bom[:])

        # repack via DRAM bounce (row p of [108,256] = flat 256p): y/x rows
        # into [36, 2, 256] (free-dim y/x), mask rows into [36, 256] -- all
        # tiles base partition 0 (engines need 32-aligned partition bases).
        nc.sync.dma_start(obounce.ap(), t_om27[:])
        t_yx = geom.tile([36, 2, 256], DT.float32, name="yx")
        nc.sync.dma_start(t_yx[:],
                          bass.AP(obounce, 0, [[256, 36], [9216, 2], [1, 256]]))
        t_m = geom.tile([36, 256], DT.float32, name="m")
        nc.sync.dma_start(t_m[:], bass.AP(obounce, 72 * 256, [[256, 36], [1, 256]]))

        # ---- geometry (packed [36, *, 256], fp32) ---------------------------
        t_ysxs = geom.tile([36, 2, 256], DT.float32, name="ysxs")
        nc.vector.tensor_add(t_ysxs[:], t_yx[:], t_baseyx[:])
        t_r = geom.tile([36, 2, 256], DT.float32, name="rtmp")
        nc.vector.tensor_scalar(t_r[:], t_ysxs[:], MAGIC, -MAGIC, ALU.add, ALU.add)
        t_gt = geom.tile([36, 2, 256], DT.float32, name="gttmp")
        nc.vector.tensor_tensor(t_gt[:], t_r[:], t_ysxs[:], ALU.is_gt)
        t_fl = geom.tile([36, 2, 256], DT.float32, name="fl")
        nc.vector.tensor_sub(t_fl[:], t_r[:], t_gt[:])

        # gather index first (unblocks the gathers asap):
        # idx = clip(y0+PADG, 0, G-1)*G + clip(x0+PADG, 0, G-1)
        t_yp = geom.tile([36, 256], DT.float32, name="yp")
        nc.vector.tensor_scalar(t_yp[:], t_fl[:, 0], float(PADG), 0.0,
                                ALU.add, ALU.max)
        t_ypg = geom.tile([36, 256], DT.float32, name="ypg")
        nc.vector.tensor_scalar(t_ypg[:], t_yp[:], float(G - 1), float(G),
                                ALU.min, ALU.mult)
        t_xp = geom.tile([36, 256], DT.float32, name="yp")
        nc.vector.tensor_scalar(t_xp[:], t_fl[:, 1], float(PADG), 0.0,
                                ALU.add, ALU.max)
        t_xpc = geom.tile([36, 256], DT.float32, name="ypg2")
        nc.vector.tensor_scalar(t_xpc[:], t_xp[:], float(G - 1), None, ALU.min)
        t_idx = geom.tile([36, 256], DT.float32, name="idx")
        nc.vector.tensor_add(t_idx[:], t_ypg[:], t_xpc[:])

        # export pre-wrapped: ibounce[a, p*16 + b] = idx[p, a*16 + b].
        # gather column j (within tap k) = lane j%16 = a, slot j//16 = qq*16+b,
        # i.e. column (qq, b, a) holds pixel (qq, a, b) -- an a<->b swap within
        # each 256-pixel block that the lambda planes and host unswap mirror.
        nc.gpsimd.dma_start(
            bass.AP(ibounce, 0, [[16, 36], [576, 16], [1, 16]]),
            t_idx[:].rearrange("p (a b) -> p a b", a=16, b=16))
        t_widx = consts.tile([128, 576], DT.int16)
        for g in range(8):
            nc.sync.dma_start(t_widx[16 * g:16 * (g + 1), :], ibounce.ap())

        # lambda planes [36, 3, 256] fp16: lx | lam0=m-m*ly | lam1=m*ly
        t_fr = geom.tile([36, 2, 256], DT.float32, name="fr")
        nc.vector.tensor_sub(t_fr[:], t_ysxs[:], t_fl[:])
        t_mask = geom.tile([36, 256], DT.float32, name="mask")
        nc.scalar.activation(t_mask[:], t_m[:], AF.Sigmoid)
        t_l1 = geom.tile([36, 256], DT.float32, name="l1")
        nc.vector.tensor_mul(t_l1[:], t_mask[:], t_fr[:, 0])
        t_l0 = geom.tile([36, 256], DT.float32, name="l0")
        nc.vector.tensor_sub(t_l0[:], t_mask[:], t_l1[:])
        # write planes a<->b swapped to match gather column order
        t_lam = geom.tile([36, 3, 256], DT.float16, name="lam")

        def swapw(dst, src):
            nc.vector.tensor_copy(
                dst.rearrange("p (b a) -> p a b", b=16, a=16),
                src.rearrange("p (a b) -> p a b", a=16, b=16))

        swapw(t_lam[:, 0], t_fr[:, 1])
        swapw(t_lam[:, 1], t_l0[:])
        swapw(t_lam[:, 2], t_l1[:])

        # ---- per-tap: gather, planes, combine, matmul -----------------------
        t_osum = [opsum.tile([128, T], DT.float32, name=f"osum{oh}")
                  for oh in range(2)]
        for k in range(K2):
            pl = planes.tile([128, 3, T], DT.float16, name="plane")
            for qq in range(4):
                pp = plpsum.tile([128, 768], DT.float32, tag="pl")
                lam_flat = t_lam[:].rearrange("p c f -> p (c f)")
                lhsT = t_sel[:, 128 * (4 * k + qq):128 * (4 * k + qq + 1)]
                # matmul out must stay within one 2KB PSUM bank (<=512 f32)
                nc.tensor.matmul(pp[:, 0:512], lhsT, lam_flat[:, 0:512],
                                 start=True, stop=True)
                nc.tensor.matmul(pp[:, 512:768], lhsT, lam_flat[:, 512:768],
                                 start=True, stop=True)
                nc.scalar.activation(pl[:, :, 256 * qq:256 * (qq + 1)],
                                     pp[:].rearrange("p (c f) -> p c f", c=3),
                                     AF.Copy)

            for hh in range(2):
                gc = gbuf.tile([128, 8, 512], DT.float16, name="gath")
                nc.gpsimd.dma_gather(
                    out_ap=gc[:],
                    in_ap=table.ap(),
                    idxs_ap=t_widx[:, 64 * k + 32 * hh:64 * k + 32 * (hh + 1)],
                    num_idxs=512,
                    num_idxs_reg=512,
                    elem_size=1024,
                    transpose=True,
                )
                sl = slice(512 * hh, 512 * (hh + 1))
                # t = lx * B; R = A + t; v = [l0,l0,l1,l1] * R
                tt = cbuf.tile([128, 4, 512], DT.float16, name="tt")
                nc.vector.tensor_mul(
                    tt[:], gc[:, 4:8],
                    pl[:, 0, sl].unsqueeze(1).broadcast_to([128, 4, 512]))
                rr = cbuf.tile([128, 4, 512], DT.float16, name="rr")
                nc.vector.tensor_add(rr[:], gc[:, 0:4], tt[:])
                vv = cbuf.tile([128, 4, 512], DT.float16, name="vv", tag="tt")
                nc.vector.tensor_mul(
                    vv[:].rearrange("p (r h) f -> p r h f", r=2, h=2),
                    rr[:].rearrange("p (r h) f -> p r h f", r=2, h=2),
                    pl[:, 1:3, sl].unsqueeze(2).broadcast_to([128, 2, 2, 512]))

                for vb in range(4):
                    lhsT = t_wdT[:, 2 * k + (vb % 2)]       # [128, 256]
                    for oh in range(2):
                        nc.tensor.matmul(
                            t_osum[oh][:, sl],
                            lhsT[:, 128 * oh:128 * (oh + 1)],
                            vv[:, vb],
                            start=(k == 0 and vb == 0),
                            stop=(k == K2 - 1 and vb == 3),
                        )

        # ---- output ---------------------------------------------------------
        for oh in range(2):
            osb = planes.tile([128, T], DT.float32, name="osb")
            nc.scalar.activation(osb[:], t_osum[oh][:], AF.Identity,
                                 bias=t_bdcn[:, oh:oh + 1])
            nc.sync.dma_start(out.ap()[128 * oh:128 * (oh + 1)], osb[:])

    nc.compile()
    return nc


def _host_prep(x, offset_feat, w_offset_mask, b_offset_mask, w_dcn, b_dcn):
    bf16 = ml_dtypes.bfloat16
    perm = list(range(0, 18, 2)) + list(range(1, 18, 2)) + list(range(18, 27))
    w_om_p = w_offset_mask[perm].astype(np.float32)      # [27, 256, 3, 3]
    b_om_p = b_offset_mask[perm].astype(np.float32)

    # womT[k*2+ch] = [128 c, 27] for tap k, channel half ch
    womT = np.zeros((18, 128, 27), np.float32)
    for k in range(9):
        ky, kx = k // 3, k % 3
        wt = w_om_p[:, :, ky, kx]                        # [27, 256]
        for ch in range(2):
            womT[2 * k + ch] = wt[:, 128 * ch:128 * (ch + 1)].T
    womT = womT.astype(bf16)

    # wdT[k*2+ch] = [128 c, 256 o]
    wd = w_dcn.reshape(C, C, 9).astype(np.float32)       # [o, c, k]
    wdT = np.zeros((18, 128, 256), np.float32)
    for k in range(9):
        for ch in range(2):
            wdT[2 * k + ch] = wd[:, 128 * ch:128 * (ch + 1), k].T
    wdT = wdT.astype(np.float16)

    bdcn = b_dcn.astype(np.float32).reshape(2, 128).T.copy()  # [128, 2]

    # pairs tables per batch: row = [A0 | A1 | B0 | B1]
    tables = []
    for b in range(B):
        xp = np.zeros((C, G + 1, G + 1), np.float32)
        xp[:, PADG:PADG + H, PADG:PADG + W] = x[b]
        A0 = xp[:, :G, :G]
        A1 = xp[:, 1:, :G]
        B0 = xp[:, :G, 1:] - A0
        B1 = xp[:, 1:, 1:] - A1
        tab = np.concatenate(
            [t.reshape(C, NROWS).T for t in (A0, A1, B0, B1)], axis=1)
        tables.append(np.ascontiguousarray(tab.astype(np.float16)))

    # packed base grids [72, 256] per h-quarter: rows (4k+qq)
    ky = np.repeat(np.arange(3), 3).astype(np.float32)   # [9]
    kx = np.tile(np.arange(3), 3).astype(np.float32)
    f = np.arange(256)
    selv = np.zeros((36, 36 * 128), np.float16)
    for r in range(36):
        selv[r, 128 * r:128 * (r + 1)] = 1.0
    in_maps = []
    for b in range(B):
        for q in range(4):
            ho0 = q * HQ
            slab = np.zeros((C, 18, 66), np.float32)
            r0 = ho0 - 1
            rr0, rr1 = max(r0, 0), min(ho0 + 17, H)
            slab[:, rr0 - r0:rr1 - r0, 1:65] = offset_feat[b][:, rr0:rr1, :]
            byx = np.zeros((36, 2, 256), np.float32)
            for k in range(9):
                for qq in range(4):
                    i = qq * 256 + f
                    byx[4 * k + qq, 0] = ho0 + i // 64 + ky[k] - 1.0
                    byx[4 * k + qq, 1] = i % 64 + kx[k] - 1.0
            in_maps.append({
                "slab": slab.astype(bf16),
                "womT": womT,
                "bom": b_om_p.reshape(27, 1),
                "wdT": wdT,
                "bdcn": bdcn,
                "baseyx": byx,
                "table": tables[b],
                "sel": selv,
            })
    return in_maps


def _get_module():
    if "nc" not in _cache:
        _cache["nc"] = _build_module()
    return _cache["nc"]


def kernel(x, offset_feat, w_offset_mask, b_offset_mask, w_dcn, b_dcn,
           **run_kwargs):
    x = np.asarray(x); offset_feat = np.asarray(offset_feat)
    w_offset_mask = np.asarray(w_offset_mask)
    b_offset_mask = np.asarray(b_offset_mask)
    w_dcn = np.asarray(w_dcn); b_dcn = np.asarray(b_dcn)

    nc = _get_module()
    in_maps = _host_prep(x, offset_feat, w_offset_mask, b_offset_mask,
                         w_dcn, b_dcn)
    res = run_bass_kernel_spmd(nc, in_maps, list(range(N_CORES)), **run_kwargs)
    y = np.zeros((B, C, H, W), np.float32)
    for ci, (b, q) in enumerate([(b, q) for b in range(B) for q in range(4)]):
        o = res.results[ci]["out"].reshape(C, 4, 16, 16)
        y[b, :, q * HQ:(q + 1) * HQ, :] = \
            o.transpose(0, 1, 3, 2).reshape(C, HQ, W)
    kernel.last_results = res
    return y


# revision 19
# speedup vs baseline: 2.8775x; 1.0358x over previous
"""DCNv2 (modulated deformable conv) Trainium2 Bass kernel.

Sharding: 8 cores = (batch b in 2) x (H-quarter q in 4); each core computes
out[b, :, 16q:16q+16, :] (256 out-channels x 1024 pixels).

Algorithm per core (v2):
  1. offset/mask 3x3 conv on an 18-row slab -> om [27, 1024] (bf16 PE matmuls).
  2. om repacked [27,1024] -> [108, 256] via DRAM bounce so geometry runs on
     ~108 partitions instead of 9 (8x fewer DVE cycles).
  3. geometry: ys/xs, floor via magic, fractions, sigmoid mask; lambda planes
     lx | lam0=m*(1-ly) | lam1=m*ly in one [36, 3, 256] fp16 tile; gather
     indices written to DRAM PRE-WRAPPED ([16, 576] int16) so the index
     readback is contiguous (the old strided readback burned ~30us of DMA).
  4. per tap: ONE dma_gather (1024 idx, 2KB rows) from a host-built "pairs"
     table row = [A0|A1|B0|B1] (A=value at y0/y0+1, B=x-neighbor diff).
  5. combine in 3 DVE ops/tap: t = lx*B(4 planes), R = A + t,
     v = [lam0,lam0,lam1,lam1] * R  -> 512-channel GEMM rhs.
  6. main conv: osum[o, p] += sum_{tap, 4 vblocks} wdcnT^T @ v (fp16 PE).
"""
import os
import sys
import numpy as np

sys.path.insert(0, "/opt/trn_rl_repo")

from contextlib import ExitStack
import ml_dtypes
import concourse.bass as bass
import concourse.bacc as bacc
import concourse.tile as tile
import concourse.mybir as mybir
from concourse.bass_utils import run_bass_kernel_spmd


AF = mybir.ActivationFunctionType
ALU = mybir.AluOpType
DT = mybir.dt

B, C, H, W, K2 = 2, 256, 64, 64, 9
PADG = 6
G = H + 2 * PADG            # 76 padded grid side
NROWS = G * G               # 5776 table rows
HQ = 16                     # output rows per core
T = HQ * W                  # 1024 pixels per core
N_CORES = 8
MAGIC = 12582912.0          # 1.5 * 2^23 round-to-int magic

_cache = {}


def _build_module():
    nc = bacc.Bacc("TRN2", debug=False, num_devices=N_CORES,
                   dynamic_dma_scratch_size=49152, num_swdge_queues=4)

    # ---- dram tensors -------------------------------------------------------
    slab = nc.dram_tensor("slab", [C, 18, 66], DT.bfloat16, kind="ExternalInput")
    womT = nc.dram_tensor("womT", [18, 128, 27], DT.bfloat16, kind="ExternalInput")
    bom = nc.dram_tensor("bom", [27, 1], DT.float32, kind="ExternalInput")
    wdT = nc.dram_tensor("wdT", [18, 128, 256], DT.float16, kind="ExternalInput")
    bdcn = nc.dram_tensor("bdcn", [128, 2], DT.float32, kind="ExternalInput")
    baseyx = nc.dram_tensor("baseyx", [36, 2, 256], DT.float32, kind="ExternalInput")
    table = nc.dram_tensor("table", [NROWS, 1024], DT.float16, kind="ExternalInput")
    sel = nc.dram_tensor("sel", [36, 36 * 128], DT.float16, kind="ExternalInput")
    out = nc.dram_tensor("out", [C, T], DT.float32, kind="ExternalOutput")
    ibounce = nc.dram_tensor("ibounce", [16, 576], DT.int16)
    obounce = nc.dram_tensor("obounce", [108, 256], DT.float32)

    with tile.TileContext(nc) as tc, ExitStack() as ctx:
        consts = ctx.enter_context(tc.tile_pool(name="consts", bufs=1))
        geom = ctx.enter_context(tc.tile_pool(name="geom", bufs=1))
        plpsum = ctx.enter_context(tc.tile_pool(name="plpsum", bufs=2,
                                                space="PSUM"))
        opsum = ctx.enter_context(tc.tile_pool(name="opsum", bufs=1,
                                               space="PSUM"))
        planes = ctx.enter_context(tc.tile_pool(name="planes", bufs=2))
        gbuf = ctx.enter_context(tc.tile_pool(name="gbuf", bufs=3))
        cbuf = ctx.enter_context(tc.tile_pool(name="cbuf", bufs=2))

        # ---- load constants -------------------------------------------------
        t_slab = []
        for ch in range(2):
            s = consts.tile([128, 18, 66], DT.bfloat16, name=f"slab{ch}")
            nc.sync.dma_start(s[:], slab.ap()[128 * ch:128 * (ch + 1)])
            t_slab.append(s)
        t_womT = consts.tile([128, 18, 27], DT.bfloat16)
        nc.sync.dma_start(t_womT[:], womT.ap().transpose([1, 0, 2]))
        t_wdT = consts.tile([128, 18, 256], DT.float16)
        nc.sync.dma_start(t_wdT[:], wdT.ap().transpose([1, 0, 2]))
        t_bom = consts.tile([27, 1], DT.float32)
        nc.sync.dma_start(t_bom[:], bom.ap())
        t_bdcn = consts.tile([128, 2], DT.float32)
        nc.sync.dma_start(t_bdcn[:], bdcn.ap())
        t_baseyx = consts.tile([36, 2, 256], DT.float32)
        nc.sync.dma_start(t_baseyx[:], baseyx.ap())
        t_sel = consts.tile([36, 36 * 128], DT.float16)
        nc.sync.dma_start(t_sel[:], sel.ap())

        # ---- offset conv: om27 rows = [dy(9) | dx(9) | m(9)] ---------------
        t_om27 = geom.tile([27, T], DT.float32, name="om27")
        for nh in range(2):
            ps = plpsum.tile([27, 512], DT.float32, tag="pl")
            i = 0
            for k in range(K2):
                ky, kx = k // 3, k % 3
                for ch in range(2):
                    rhs = t_slab[ch][:, 8 * nh + ky: 8 * nh + ky + 8, kx:kx + 64]
                    nc.tensor.matmul(ps[:], t_womT[:, 2 * k + ch], rhs,
                                     start=(i == 0), stop=(i == 17))
                    i += 1
            nc.scalar.activation(t_om27[:, 512 * nh:512 * (nh + 1)], ps[:],
                                 AF.Identity, bias=t_bom[:])

        # repack via DRAM bounce (row p of [108,256] = flat 256p): y/x rows
        # into [36, 2, 256] (free-dim y/x), mask rows into [36, 256] -- all
        # tiles base partition 0 (engines need 32-aligned partition bases).
        nc.sync.dma_start(obounce.ap(), t_om27[:])
        t_yx = geom.tile([36, 2, 256], DT.float32, name="yx")
        nc.sync.dma_start(t_yx[:],
                          bass.AP(obounce, 0, [[256, 36], [9216, 2], [1, 256]]))
        t_m = geom.tile([36, 256], DT.float32, name="m")
        nc.sync.dma_start(t_m[:], bass.AP(obounce, 72 * 256, [[256, 36], [1, 256]]))

        # ---- geometry (packed [36, *, 256], fp32) ---------------------------
        t_ysxs = geom.tile([36, 2, 256], DT.float32, name="ysxs")
        nc.vector.tensor_add(t_ysxs[:], t_yx[:], t_baseyx[:])
        t_r = geom.tile([36, 2, 256], DT.float32, name="rtmp")
        nc.vector.tensor_scalar(t_r[:], t_ysxs[:], MAGIC, -MAGIC, ALU.add, ALU.add)
        t_gt = geom.tile([36, 2, 256], DT.float32, name="gttmp")
        nc.vector.tensor_tensor(t_gt[:], t_r[:], t_ysxs[:], ALU.is_gt)
        t_fl = geom.tile([36, 2, 256], DT.float32, name="fl")
        nc.vector.tensor_sub(t_fl[:], t_r[:], t_gt[:])

        # gather index first (unblocks the gathers asap):
        # idx = clip(y0+PADG, 0, G-1)*G + clip(x0+PADG, 0, G-1)
        t_yp = geom.tile([36, 256], DT.float32, name="yp")
        nc.vector.tensor_scalar(t_yp[:], t_fl[:, 0], float(PADG), 0.0,
                                ALU.add, ALU.max)
        t_ypg = geom.tile([36, 256], DT.float32, name="ypg")
        nc.vector.tensor_scalar(t_ypg[:], t_yp[:], float(G - 1), float(G),
                                ALU.min, ALU.mult)
        t_xp = geom.tile([36, 256], DT.float32, name="yp")
        nc.vector.tensor_scalar(t_xp[:], t_fl[:, 1], float(PADG), 0.0,
                                ALU.add, ALU.max)
        t_xpc = geom.tile([36, 256], DT.float32, name="ypg2")
        nc.vector.tensor_scalar(t_xpc[:], t_xp[:], float(G - 1), None, ALU.min)
        t_idx = geom.tile([36, 256], DT.float32, name="idx")
        nc.vector.tensor_add(t_idx[:], t_ypg[:], t_xpc[:])

        # export pre-wrapped: ibounce[a, p*16 + b] = idx[p, a*16 + b].
        # gather column j (within tap k) = lane j%16 = a, slot j//16 = qq*16+b,
        # i.e. column (qq, b, a) holds pixel (qq, a, b) -- an a<->b swap within
        # each 256-pixel block that the lambda planes and host unswap mirror.
        nc.gpsimd.dma_start(
            bass.AP(ibounce, 0, [[16, 36], [576, 16], [1, 16]]),
            t_idx[:].rearrange("p (a b) -> p a b", a=16, b=16))
        t_widx = consts.tile([128, 576], DT.int16)
        for g in range(8):
            nc.sync.dma_start(t_widx[16 * g:16 * (g + 1), :], ibounce.ap())

        # lambda planes [36, 3, 256] fp16: lx | lam0=m-m*ly | lam1=m*ly
        t_fr = geom.tile([36, 2, 256], DT.float32, name="fr")
        nc.vector.tensor_sub(t_fr[:], t_ysxs[:], t_fl[:])
        t_mask = geom.tile([36, 256], DT.float32, name="mask")
        nc.scalar.activation(t_mask[:], t_m[:], AF.Sigmoid)
        t_l1 = geom.tile([36, 256], DT.float32, name="l1")
        nc.vector.tensor_mul(t_l1[:], t_mask[:], t_fr[:, 0])
        t_l2 = geom.tile([36, 256], DT.float32, name="l2")
        nc.vector.tensor_mul(t_l2[:], t_mask[:], t_fr[:, 1])
        t_l3 = geom.tile([36, 256], DT.float32, name="l3")
        nc.vector.tensor_mul(t_l3[:], t_l1[:], t_fr[:, 1])
        # write planes a<->b swapped to match gather column order
        t_lam = geom.tile([36, 4, 256], DT.float16, name="lam")

        def swapw(dst, src):
            nc.vector.tensor_copy(
                dst.rearrange("p (b a) -> p a b", b=16, a=16),
                src.rearrange("p (a b) -> p a b", a=16, b=16))

        swapw(t_lam[:, 0], t_mask[:])
        swapw(t_lam[:, 1], t_l1[:])
        swapw(t_lam[:, 2], t_l2[:])
        swapw(t_lam[:, 3], t_l3[:])

        # ---- per-tap: gather, planes, combine, matmul -----------------------
        t_osum = [opsum.tile([128, T], DT.float32, name=f"osum{oh}")
                  for oh in range(2)]
        for k in range(K2):
            pl = planes.tile([128, 4, T], DT.float16, name="plane")
            for qq in range(4):
                pp = plpsum.tile([128, 1024], DT.float32, tag="pl")
                lam_flat = t_lam[:].rearrange("p c f -> p (c f)")
                lhsT = t_sel[:, 128 * (4 * k + qq):128 * (4 * k + qq + 1)]
                # matmul out must stay within one 2KB PSUM bank (<=512 f32)
                nc.tensor.matmul(pp[:, 0:512], lhsT, lam_flat[:, 0:512],
                                 start=True, stop=True)
                nc.tensor.matmul(pp[:, 512:1024], lhsT, lam_flat[:, 512:1024],
                                 start=True, stop=True)
                nc.scalar.activation(pl[:, :, 256 * qq:256 * (qq + 1)],
                                     pp[:].rearrange("p (c f) -> p c f", c=4),
                                     AF.Copy)

            vals = []
            for hh in range(2):
                gc = gbuf.tile([128, 8, 512], DT.float16, name="gath")
                nc.gpsimd.dma_gather(
                    out_ap=gc[:],
                    in_ap=table.ap(),
                    idxs_ap=t_widx[:, 64 * k + 32 * hh:64 * k + 32 * (hh + 1)],
                    num_idxs=512,
                    num_idxs_reg=512,
                    elem_size=1024,
                    transpose=True,
                    queue_num=(2 * k + hh) % 4,
                )
                sl = slice(512 * hh, 512 * (hh + 1))
                # P = lam * T (4 planes); val = (P1+Py) + (Px+Pxy)
                ptt = cbuf.tile([128, 8, 512], DT.float16, name="ptt")
                nc.vector.tensor_mul(
                    ptt[:].rearrange("p (c h) f -> p c h f", c=4, h=2),
                    gc[:].rearrange("p (c h) f -> p c h f", c=4, h=2),
                    pl[:, :, sl].unsqueeze(2).broadcast_to([128, 4, 2, 512]))
                aa = cbuf.tile([128, 2, 512], DT.float16, name="aa")
                nc.vector.tensor_add(aa[:], ptt[:, 0:2], ptt[:, 2:4])
                bb = cbuf.tile([128, 2, 512], DT.float16, name="bb")
                nc.vector.tensor_add(bb[:], ptt[:, 4:6], ptt[:, 6:8])
                vv = cbuf.tile([128, 2, 512], DT.float16, name=f"vv{hh}")
                nc.vector.tensor_add(vv[:], aa[:], bb[:])
                vals.append(vv)

            for ch in range(2):
                for oh in range(2):
                    lhsT = t_wdT[:, 2 * k + ch][:, 128 * oh:128 * (oh + 1)]
                    for hh in range(2):
                        nc.tensor.matmul(
                            t_osum[oh][:, 512 * hh:512 * (hh + 1)],
                            lhsT,
                            vals[hh][:, ch],
                            start=(k == 0 and ch == 0),
                            stop=(k == K2 - 1 and ch == 1),
                        )

        # ---- output ---------------------------------------------------------
        for oh in range(2):
            osb = planes.tile([128, T], DT.float32, name="osb")
            nc.scalar.activation(osb[:], t_osum[oh][:], AF.Identity,
                                 bias=t_bdcn[:, oh:oh + 1])
            nc.sync.dma_start(out.ap()[128 * oh:128 * (oh + 1)], osb[:])

    nc.compile()
    return nc


def _host_prep(x, offset_feat, w_offset_mask, b_offset_mask, w_dcn, b_dcn):
    bf16 = ml_dtypes.bfloat16
    perm = list(range(0, 18, 2)) + list(range(1, 18, 2)) + list(range(18, 27))
    w_om_p = w_offset_mask[perm].astype(np.float32)      # [27, 256, 3, 3]
    b_om_p = b_offset_mask[perm].astype(np.float32)

    # womT[k*2+ch] = [128 c, 27] for tap k, channel half ch
    womT = np.zeros((18, 128, 27), np.float32)
    for k in range(9):
        ky, kx = k // 3, k % 3
        wt = w_om_p[:, :, ky, kx]                        # [27, 256]
        for ch in range(2):
            womT[2 * k + ch] = wt[:, 128 * ch:128 * (ch + 1)].T
    womT = womT.astype(bf16)

    # wdT[k*2+ch] = [128 c, 256 o]
    wd = w_dcn.reshape(C, C, 9).astype(np.float32)       # [o, c, k]
    wdT = np.zeros((18, 128, 256), np.float32)
    for k in range(9):
        for ch in range(2):
            wdT[2 * k + ch] = wd[:, 128 * ch:128 * (ch + 1), k].T
    wdT = wdT.astype(np.float16)

    bdcn = b_dcn.astype(np.float32).reshape(2, 128).T.copy()  # [128, 2]

    # pairs tables per batch: row = [A0 | A1 | B0 | B1]
    tables = []
    for b in range(B):
        xp = np.zeros((C, G + 1, G + 1), np.float32)
        xp[:, PADG:PADG + H, PADG:PADG + W] = x[b]
        T1 = xp[:, :G, :G]
        Ty = xp[:, 1:, :G] - T1
        Tx = xp[:, :G, 1:] - T1
        Txy = xp[:, 1:, 1:] - xp[:, 1:, :G] - xp[:, :G, 1:] + T1
        tab = np.concatenate(
            [t.reshape(C, NROWS).T for t in (T1, Ty, Tx, Txy)], axis=1)
        tables.append(np.ascontiguousarray(tab.astype(np.float16)))

    # packed base grids [72, 256] per h-quarter: rows (4k+qq)
    ky = np.repeat(np.arange(3), 3).astype(np.float32)   # [9]
    kx = np.tile(np.arange(3), 3).astype(np.float32)
    f = np.arange(256)
    selv = np.zeros((36, 36 * 128), np.float16)
    for r in range(36):
        selv[r, 128 * r:128 * (r + 1)] = 1.0
    in_maps = []
    for b in range(B):
        for q in range(4):
            ho0 = q * HQ
            slab = np.zeros((C, 18, 66), np.float32)
            r0 = ho0 - 1
            rr0, rr1 = max(r0, 0), min(ho0 + 17, H)
            slab[:, rr0 - r0:rr1 - r0, 1:65] = offset_feat[b][:, rr0:rr1, :]
            byx = np.zeros((36, 2, 256), np.float32)
            for k in range(9):
                for qq in range(4):
                    i = qq * 256 + f
                    byx[4 * k + qq, 0] = ho0 + i // 64 + ky[k] - 1.0
                    byx[4 * k + qq, 1] = i % 64 + kx[k] - 1.0
            in_maps.append({
                "slab": slab.astype(bf16),
                "womT": womT,
                "bom": b_om_p.reshape(27, 1),
                "wdT": wdT,
                "bdcn": bdcn,
                "baseyx": byx,
                "table": tables[b],
                "sel": selv,
            })
    return in_maps


def _get_module():
    if "nc" not in _cache:
        _cache["nc"] = _build_module()
    return _cache["nc"]


def kernel(x, offset_feat, w_offset_mask, b_offset_mask, w_dcn, b_dcn,
           **run_kwargs):
    x = np.asarray(x); offset_feat = np.asarray(offset_feat)
    w_offset_mask = np.asarray(w_offset_mask)
    b_offset_mask = np.asarray(b_offset_mask)
    w_dcn = np.asarray(w_dcn); b_dcn = np.asarray(b_dcn)

    nc = _get_module()
    in_maps = _host_prep(x, offset_feat, w_offset_mask, b_offset_mask,
                         w_dcn, b_dcn)
    res = run_bass_kernel_spmd(nc, in_maps, list(range(N_CORES)), **run_kwargs)
    y = np.zeros((B, C, H, W), np.float32)
    for ci, (b, q) in enumerate([(b, q) for b in range(B) for q in range(4)]):
        o = res.results[ci]["out"].reshape(C, 4, 16, 16)
        y[b, :, q * HQ:(q + 1) * HQ, :] = \
            o.transpose(0, 1, 3, 2).reshape(C, HQ, W)
    kernel.last_results = res
    return y


# revision 31
# speedup vs baseline: 3.0489x; 1.0596x over previous
"""DCNv2 (modulated deformable conv) Trainium2 Bass kernel.

Sharding: 8 cores = (batch b in 2) x (H-quarter q in 4); each core computes
out[b, :, 16q:16q+16, :] (256 out-channels x 1024 pixels).

Algorithm per core (v2):
  1. offset/mask 3x3 conv on an 18-row slab -> om [27, 1024] (bf16 PE matmuls).
  2. om repacked [27,1024] -> [108, 256] via DRAM bounce so geometry runs on
     ~108 partitions instead of 9 (8x fewer DVE cycles).
  3. geometry: ys/xs, floor via magic, fractions, sigmoid mask; lambda planes
     lx | lam0=m*(1-ly) | lam1=m*ly in one [36, 3, 256] fp16 tile; gather
     indices written to DRAM PRE-WRAPPED ([16, 576] int16) so the index
     readback is contiguous (the old strided readback burned ~30us of DMA).
  4. per tap: ONE dma_gather (1024 idx, 2KB rows) from a host-built "pairs"
     table row = [A0|A1|B0|B1] (A=value at y0/y0+1, B=x-neighbor diff).
  5. combine in 3 DVE ops/tap: t = lx*B(4 planes), R = A + t,
     v = [lam0,lam0,lam1,lam1] * R  -> 512-channel GEMM rhs.
  6. main conv: osum[o, p] += sum_{tap, 4 vblocks} wdcnT^T @ v (fp16 PE).
"""
import os
import sys
import numpy as np

sys.path.insert(0, "/opt/trn_rl_repo")

from contextlib import ExitStack
import ml_dtypes
import concourse.bass as bass
import concourse.bacc as bacc
import concourse.tile as tile
import concourse.mybir as mybir
from concourse.bass_utils import run_bass_kernel_spmd


AF = mybir.ActivationFunctionType
ALU = mybir.AluOpType
DT = mybir.dt

B, C, H, W, K2 = 2, 256, 64, 64, 9
PADG = 6
G = H + 2 * PADG            # 76 padded grid side
NROWS = G * G               # 5776 table rows
HQ = 16                     # output rows per core
T = HQ * W                  # 1024 pixels per core
N_CORES = 8
MAGIC = 12582912.0          # 1.5 * 2^23 round-to-int magic

_cache = {}


def _build_module():
    nc = bacc.Bacc("TRN2", debug=False, num_devices=N_CORES,
                   dynamic_dma_scratch_size=49152, num_swdge_queues=4)

    # ---- dram tensors -------------------------------------------------------
    slab = nc.dram_tensor("slab", [C, 18, 66], DT.bfloat16, kind="ExternalInput")
    womT = nc.dram_tensor("womT", [18, 128, 27], DT.bfloat16, kind="ExternalInput")
    bom = nc.dram_tensor("bom", [27, 1], DT.float32, kind="ExternalInput")
    wdT = nc.dram_tensor("wdT", [18, 128, 256], DT.float16, kind="ExternalInput")
    bdcn = nc.dram_tensor("bdcn", [128, 2], DT.float32, kind="ExternalInput")
    baseyx = nc.dram_tensor("baseyx", [36, 2, 256], DT.float32, kind="ExternalInput")
    table = nc.dram_tensor("table", [NROWS, 1024], DT.float16, kind="ExternalInput")
    sel = nc.dram_tensor("sel", [36, 36 * 128], DT.float16, kind="ExternalInput")
    out = nc.dram_tensor("out", [C, T], DT.float32, kind="ExternalOutput")
    ibounce = nc.dram_tensor("ibounce", [16, 576], DT.int16)
    obounce = nc.dram_tensor("obounce", [108, 256], DT.float32)

    with tile.TileContext(nc) as tc, ExitStack() as ctx:
        consts = ctx.enter_context(tc.tile_pool(name="consts", bufs=1))
        geom = ctx.enter_context(tc.tile_pool(name="geom", bufs=1))
        plpsum = ctx.enter_context(tc.tile_pool(name="plpsum", bufs=2,
                                                space="PSUM"))
        opsum = ctx.enter_context(tc.tile_pool(name="opsum", bufs=1,
                                               space="PSUM"))
        planes = ctx.enter_context(tc.tile_pool(name="planes", bufs=3))
        gbuf = ctx.enter_context(tc.tile_pool(name="gbuf", bufs=3))
        cbuf = ctx.enter_context(tc.tile_pool(name="cbuf", bufs=2))

        # ---- load constants -------------------------------------------------
        t_slab = []
        for ch in range(2):
            s = consts.tile([128, 18, 66], DT.bfloat16, name=f"slab{ch}")
            nc.sync.dma_start(s[:], slab.ap()[128 * ch:128 * (ch + 1)])
            t_slab.append(s)
        t_womT = consts.tile([128, 18, 27], DT.bfloat16)
        nc.sync.dma_start(t_womT[:], womT.ap().transpose([1, 0, 2]))
        t_bom = consts.tile([27, 1], DT.float32)
        nc.sync.dma_start(t_bom[:], bom.ap())
        t_baseyx = consts.tile([36, 2, 256], DT.float32)
        nc.sync.dma_start(t_baseyx[:], baseyx.ap())
        t_bdcn = consts.tile([128, 2], DT.float32)
        nc.sync.dma_start(t_bdcn[:], bdcn.ap())
        t_wdT = consts.tile([128, 18, 256], DT.float16)
        nc.sync.dma_start(t_wdT[:], wdT.ap().transpose([1, 0, 2]))
        t_sel = consts.tile([36, 36 * 128], DT.float16)
        nc.sync.dma_start(t_sel[:], sel.ap())

        # ---- offset conv: om27 rows = [dy(9) | dx(9) | m(9)] ---------------
        t_om27 = geom.tile([27, T], DT.float32, name="om27")
        for nh in range(2):
            ps = plpsum.tile([27, 512], DT.float32, tag="pl")
            i = 0
            for k in range(K2):
                ky, kx = k // 3, k % 3
                for ch in range(2):
                    rhs = t_slab[ch][:, 8 * nh + ky: 8 * nh + ky + 8, kx:kx + 64]
                    nc.tensor.matmul(ps[:], t_womT[:, 2 * k + ch], rhs,
                                     start=(i == 0), stop=(i == 17))
                    i += 1
            nc.scalar.activation(t_om27[:, 512 * nh:512 * (nh + 1)], ps[:],
                                 AF.Identity, bias=t_bom[:])

        # repack via DRAM bounce (row p of [108,256] = flat 256p): y/x rows
        # into [36, 2, 256] (free-dim y/x), mask rows into [36, 256] -- all
        # tiles base partition 0 (engines need 32-aligned partition bases).
        nc.sync.dma_start(obounce.ap(), t_om27[:])
        t_yx = geom.tile([36, 2, 256], DT.float32, name="yx")
        nc.sync.dma_start(t_yx[:],
                          bass.AP(obounce, 0, [[256, 36], [9216, 2], [1, 256]]))
        t_m = geom.tile([36, 256], DT.float32, name="m")
        nc.sync.dma_start(t_m[:], bass.AP(obounce, 72 * 256, [[256, 36], [1, 256]]))

        # ---- geometry (packed [36, *, 256], fp32) ---------------------------
        t_ysxs = geom.tile([36, 2, 256], DT.float32, name="ysxs")
        nc.vector.tensor_add(t_ysxs[:], t_yx[:], t_baseyx[:])
        t_r = geom.tile([36, 2, 256], DT.float32, name="rtmp")
        nc.vector.tensor_scalar(t_r[:], t_ysxs[:], MAGIC, -MAGIC, ALU.add, ALU.add)
        t_gt = geom.tile([36, 2, 256], DT.float32, name="gttmp")
        nc.vector.tensor_tensor(t_gt[:], t_r[:], t_ysxs[:], ALU.is_gt)
        t_fl = geom.tile([36, 2, 256], DT.float32, name="fl")
        nc.vector.tensor_sub(t_fl[:], t_r[:], t_gt[:])

        # gather index first (unblocks the gathers asap):
        # idx = clip(y0+PADG, 0, G-1)*G + clip(x0+PADG, 0, G-1)
        t_yp = geom.tile([36, 256], DT.float32, name="yp")
        nc.vector.tensor_scalar(t_yp[:], t_fl[:, 0], float(PADG), 0.0,
                                ALU.add, ALU.max)
        t_ypg = geom.tile([36, 256], DT.float32, name="ypg")
        nc.vector.tensor_scalar(t_ypg[:], t_yp[:], float(G - 1), float(G),
                                ALU.min, ALU.mult)
        t_xp = geom.tile([36, 256], DT.float32, name="yp")
        nc.vector.tensor_scalar(t_xp[:], t_fl[:, 1], float(PADG), 0.0,
                                ALU.add, ALU.max)
        t_xpc = geom.tile([36, 256], DT.float32, name="ypg2")
        nc.vector.tensor_scalar(t_xpc[:], t_xp[:], float(G - 1), None, ALU.min)
        t_idx = geom.tile([36, 256], DT.float32, name="idx")
        nc.vector.tensor_add(t_idx[:], t_ypg[:], t_xpc[:])

        # export pre-wrapped: ibounce[a, p*16 + b] = idx[p, a*16 + b].
        # gather column j (within tap k) = lane j%16 = a, slot j//16 = qq*16+b,
        # i.e. column (qq, b, a) holds pixel (qq, a, b) -- an a<->b swap within
        # each 256-pixel block that the lambda planes and host unswap mirror.
        nc.gpsimd.dma_start(
            bass.AP(ibounce, 0, [[16, 36], [576, 16], [1, 16]]),
            t_idx[:].rearrange("p (a b) -> p a b", a=16, b=16))
        t_widx = consts.tile([128, 576], DT.int16)
        for g in range(8):
            nc.sync.dma_start(t_widx[16 * g:16 * (g + 1), :], ibounce.ap())

        # lambda planes [36, 3, 256] fp16: lx | lam0=m-m*ly | lam1=m*ly
        t_fr = geom.tile([36, 2, 256], DT.float32, name="fr")
        nc.vector.tensor_sub(t_fr[:], t_ysxs[:], t_fl[:])
        t_mask = geom.tile([36, 256], DT.float32, name="mask")
        nc.scalar.activation(t_mask[:], t_m[:], AF.Sigmoid)
        t_l1 = geom.tile([36, 256], DT.float32, name="l1")
        nc.vector.tensor_mul(t_l1[:], t_mask[:], t_fr[:, 0])
        t_l2 = geom.tile([36, 256], DT.float32, name="l2")
        nc.vector.tensor_mul(t_l2[:], t_mask[:], t_fr[:, 1])
        t_l3 = geom.tile([36, 256], DT.float32, name="l3")
        nc.vector.tensor_mul(t_l3[:], t_l1[:], t_fr[:, 1])
        # write planes a<->b swapped to match gather column order
        t_lam = geom.tile([36, 4, 256], DT.float16, name="lam")

        def swapw(dst, src):
            nc.vector.tensor_copy(
                dst.rearrange("p (b a) -> p a b", b=16, a=16),
                src.rearrange("p (a b) -> p a b", a=16, b=16))

        swapw(t_lam[:, 0], t_mask[:])
        swapw(t_lam[:, 1], t_l1[:])
        swapw(t_lam[:, 2], t_l2[:])
        swapw(t_lam[:, 3], t_l3[:])

        # ---- per-tap: gather, planes, combine, matmul -----------------------
        t_osum = [opsum.tile([128, T], DT.float32, name=f"osum{oh}")
                  for oh in range(2)]
        for k in range(K2):
            pl = planes.tile([128, 4, T], DT.float16, name="plane")
            for qq in range(4):
                pp = plpsum.tile([128, 1024], DT.float32, tag="pl")
                lam_flat = t_lam[:].rearrange("p c f -> p (c f)")
                lhsT = t_sel[:, 128 * (4 * k + qq):128 * (4 * k + qq + 1)]
                # matmul out must stay within one 2KB PSUM bank (<=512 f32)
                nc.tensor.matmul(pp[:, 0:512], lhsT, lam_flat[:, 0:512],
                                 start=True, stop=True)
                nc.tensor.matmul(pp[:, 512:1024], lhsT, lam_flat[:, 512:1024],
                                 start=True, stop=True)
                nc.scalar.activation(pl[:, :, 256 * qq:256 * (qq + 1)],
                                     pp[:].rearrange("p (c f) -> p c f", c=4),
                                     AF.Copy)

            vals = []
            for hh in range(2):
                qn = (2 * k + hh) % 4
                gc = gbuf.tile([128, 8, 512], DT.float16, name="gath")
                nc.gpsimd.dma_gather(
                    out_ap=gc[:],
                    in_ap=table.ap(),
                    idxs_ap=t_widx[:, 64 * k + 32 * hh:64 * k + 32 * (hh + 1)],
                    num_idxs=512,
                    num_idxs_reg=512,
                    elem_size=1024,
                    transpose=True,
                    queue_num=qn,
                )
                sl = slice(512 * hh, 512 * (hh + 1))
                # P = lam * T (4 planes); val = (P1+Py) + (Px+Pxy)
                ptt = cbuf.tile([128, 8, 512], DT.float16, name="ptt")
                nc.vector.tensor_mul(
                    ptt[:].rearrange("p (c h) f -> p c h f", c=4, h=2),
                    gc[:].rearrange("p (c h) f -> p c h f", c=4, h=2),
                    pl[:, :, sl].unsqueeze(2).broadcast_to([128, 4, 2, 512]))
                aa = cbuf.tile([128, 2, 512], DT.float16, name="aa")
                nc.vector.tensor_add(aa[:], ptt[:, 0:2], ptt[:, 2:4])
                bb = cbuf.tile([128, 2, 512], DT.float16, name="bb")
                nc.vector.tensor_add(bb[:], ptt[:, 4:6], ptt[:, 6:8])
                vv = cbuf.tile([128, 2, 512], DT.float16, name=f"vv{hh}")
                nc.vector.tensor_add(vv[:], aa[:], bb[:])
                vals.append(vv)

            for ch in range(2):
                for oh in range(2):
                    lhsT = t_wdT[:, 2 * k + ch][:, 128 * oh:128 * (oh + 1)]
                    for hh in range(2):
                        nc.tensor.matmul(
                            t_osum[oh][:, 512 * hh:512 * (hh + 1)],
                            lhsT,
                            vals[hh][:, ch],
                            start=(k == 0 and ch == 0),
                            stop=(k == K2 - 1 and ch == 1),
                        )

        # ---- output ---------------------------------------------------------
        for oh in range(2):
            osb = planes.tile([128, T], DT.float32, name="osb")
            nc.scalar.activation(osb[:], t_osum[oh][:], AF.Identity,
                                 bias=t_bdcn[:, oh:oh + 1])
            nc.sync.dma_start(out.ap()[128 * oh:128 * (oh + 1)], osb[:])

    nc.compile()
    return nc


def _host_prep(x, offset_feat, w_offset_mask, b_offset_mask, w_dcn, b_dcn):
    bf16 = ml_dtypes.bfloat16
    perm = list(range(0, 18, 2)) + list(range(1, 18, 2)) + list(range(18, 27))
    w_om_p = w_offset_mask[perm].astype(np.float32)      # [27, 256, 3, 3]
    b_om_p = b_offset_mask[perm].astype(np.float32)

    # womT[k*2+ch] = [128 c, 27] for tap k, channel half ch
    womT = np.zeros((18, 128, 27), np.float32)
    for k in range(9):
        ky, kx = k // 3, k % 3
        wt = w_om_p[:, :, ky, kx]                        # [27, 256]
        for ch in range(2):
            womT[2 * k + ch] = wt[:, 128 * ch:128 * (ch + 1)].T
    womT = womT.astype(bf16)

    # wdT[k*2+ch] = [128 c, 256 o]
    wd = w_dcn.reshape(C, C, 9).astype(np.float32)       # [o, c, k]
    wdT = np.zeros((18, 128, 256), np.float32)
    for k in range(9):
        for ch in range(2):
            wdT[2 * k + ch] = wd[:, 128 * ch:128 * (ch + 1), k].T
    wdT = wdT.astype(np.float16)

    bdcn = b_dcn.astype(np.float32).reshape(2, 128).T.copy()  # [128, 2]

    # pairs tables per batch: row = [A0 | A1 | B0 | B1]
    tables = []
    for b in range(B):
        xp = np.zeros((C, G + 1, G + 1), np.float32)
        xp[:, PADG:PADG + H, PADG:PADG + W] = x[b]
        T1 = xp[:, :G, :G]
        Ty = xp[:, 1:, :G] - T1
        Tx = xp[:, :G, 1:] - T1
        Txy = xp[:, 1:, 1:] - xp[:, 1:, :G] - xp[:, :G, 1:] + T1
        tab = np.concatenate(
            [t.reshape(C, NROWS).T for t in (T1, Ty, Tx, Txy)], axis=1)
        tables.append(np.ascontiguousarray(tab.astype(np.float16)))

    # packed base grids [72, 256] per h-quarter: rows (4k+qq)
    ky = np.repeat(np.arange(3), 3).astype(np.float32)   # [9]
    kx = np.tile(np.arange(3), 3).astype(np.float32)
    f = np.arange(256)
    selv = np.zeros((36, 36 * 128), np.float16)
    for r in range(36):
        selv[r, 128 * r:128 * (r + 1)] = 1.0
    in_maps = []
    for b in range(B):
        for q in range(4):
            ho0 = q * HQ
            slab = np.zeros((C, 18, 66), np.float32)
            r0 = ho0 - 1
            rr0, rr1 = max(r0, 0), min(ho0 + 17, H)
            slab[:, rr0 - r0:rr1 - r0, 1:65] = offset_feat[b][:, rr0:rr1, :]
            byx = np.zeros((36, 2, 256), np.float32)
            for k in range(9):
                for qq in range(4):
                    i = qq * 256 + f
                    byx[4 * k + qq, 0] = ho0 + i // 64 + ky[k] - 1.0
                    byx[4 * k + qq, 1] = i % 64 + kx[k] - 1.0
            in_maps.append({
                "slab": slab.astype(bf16),
                "womT": womT,
                "bom": b_om_p.reshape(27, 1),
                "wdT": wdT,
                "bdcn": bdcn,
                "baseyx": byx,
                "table": tables[b],
                "sel": selv,
            })
    return in_maps


def _get_module():
    if "nc" not in _cache:
        _cache["nc"] = _build_module()
    return _cache["nc"]


def kernel(x, offset_feat, w_offset_mask, b_offset_mask, w_dcn, b_dcn,
           **run_kwargs):
    x = np.asarray(x); offset_feat = np.asarray(offset_feat)
    w_offset_mask = np.asarray(w_offset_mask)
    b_offset_mask = np.asarray(b_offset_mask)
    w_dcn = np.asarray(w_dcn); b_dcn = np.asarray(b_dcn)

    nc = _get_module()
    in_maps = _host_prep(x, offset_feat, w_offset_mask, b_offset_mask,
                         w_dcn, b_dcn)
    res = run_bass_kernel_spmd(nc, in_maps, list(range(N_CORES)), **run_kwargs)
    y = np.zeros((B, C, H, W), np.float32)
    for ci, (b, q) in enumerate([(b, q) for b in range(B) for q in range(4)]):
        o = res.results[ci]["out"].reshape(C, 4, 16, 16)
        y[b, :, q * HQ:(q + 1) * HQ, :] = \
            o.transpose(0, 1, 3, 2).reshape(C, HQ, W)
    kernel.last_results = res
    return y
